# revision 1
# baseline (speedup 1.0000x reference)
"""DAWN block (moe_routing) Trainium2 kernel: 8-core SPMD.

Sharding: core c = (batch b=c//2, half h=c%2). Each core handles one batch's
attention + memory block for half the queries. Inputs are row-PERMUTED per
core (local query rows first) so the program is uniform across cores; the
causal structure comes entirely from the mask data. One tiny AllReduce
combines the memory-router pooling partials across cores.

Self-contained: only needs concourse (bass) + jax + numpy at runtime.
"""
import numpy as np
import ml_dtypes

import concourse.bass as bass
import concourse.mybir as mybir
import concourse.tile as tile
from concourse import bacc

B, S, D = 4, 1024, 1024
H, DH = 16, 64
R = 128
NEXP = 64          # n_compress == n_expand
NK, KR = 4096, 128
TOPK = 8
N_CORES = 8
SQ = S // 2        # local queries per core
P = 128
NT = S // P        # 8 s-tiles (full seq)
QT = SQ // P       # 4 local q-tiles
DT_T = D // P      # 8 d-tiles

USE_BF16 = True
ABLATE = set()
F32 = mybir.dt.float32
U8 = mybir.dt.uint8
U32 = mybir.dt.uint32
NEG = -1.0e9
AX = None  # set below
ALU = mybir.AluOpType
ACT = mybir.ActivationFunctionType


def bdt():
    return mybir.dt.bfloat16 if USE_BF16 else mybir.dt.float32


def np_bdt():
    return ml_dtypes.bfloat16 if USE_BF16 else np.float32


def build_nc(use_cc=True, dbg=False):
    global AX
    AX = mybir.AxisListType.X
    DT = bdt()
    nc = bacc.Bacc("TRN2", target_bir_lowering=False, debug=False,
                   num_devices=N_CORES)
    I = {}

    def inp(name, shape, dt):
        I[name] = nc.dram_tensor(name, shape, dt, kind="ExternalInput").ap()

    inp("x", [S, D], F32)            # row-permuted batch
    inp("imp", [S, 1], DT)           # row-permuted importance column
    inp("mT", [S, SQ], U8)           # mask transposed: [key, local query]
    inp("cn", [NEXP, D, R], DT)      # compress_neurons
    inp("pl", [NEXP, R, D], DT)      # expand pool
    inp("wct", [D, 4 * NEXP], DT)    # [Wc|WQ|WK|WV].T
    inp("wmt", [D, NEXP], DT)        # Wm.T
    inp("wot", [D, D], DT)           # WO.T
    inp("kkt", [KR, NK], DT)         # knowledge_K.T
    inp("kv", [NK, D], DT)           # knowledge_V
    inp("g1", [1, D], F32)
    inp("b1", [1, D], F32)
    inp("g2", [1, D], F32)
    inp("b2", [1, D], F32)
    inp("bselr", [1, B], F32)        # batch one-hot row
    inp("bselc", [B, 1], F32)        # batch one-hot column
    inp("idn", [P, P], DT)           # identity
    inp("idn32", [P, P], F32)        # identity fp32
    inp("bmS", [P, 32], DT)          # bmS[k,m] = (k//4 == m)
    inp("A4", [NEXP, P], F32)        # A4[j,k] = (j%4 == k%4)
    inp("B16", [NEXP, 16], F32)      # B16[j,g] = (j//4 == g)
    o = nc.dram_tensor("o", [SQ, D], F32, kind="ExternalOutput").ap()
    DBG = {}
    if dbg:
        for nm, shp, dt in [("o_wr", [1, 4 * NEXP], F32),
                            ("o_sc", [P, D], bdt()),
                            ("o_e3", [P, 3 * D], bdt()),
                            ("o_hT", [P, S], bdt()),
                            ("o_at", [P, SQ], bdt()),
                            ("o_y", [SQ, D], F32),
                            ("o_ks", [P, NK], F32),
                            ("o_mw", [1, NEXP], F32),
                            ("o_idx", [P, TOPK], mybir.dt.uint32)]:
            DBG[nm] = nc.dram_tensor(nm, shp, dt, kind="ExternalOutput").ap()

    with tile.TileContext(nc) as tc:
        _body(nc, tc, I, o, use_cc, DBG)
    nc.compile()
    return nc


def _body(nc, tc, I, o, use_cc, DBG=None):
    DT = bdt()
    esz = mybir.dt.size(DT)
    import contextlib
    ctx = contextlib.ExitStack()
    with ctx:
        pp = ctx.enter_context(tc.tile_pool(name="pers", bufs=1))
        sp = ctx.enter_context(tc.tile_pool(name="stream", bufs=2))
        # PSUM budget (8 banks): tpp 2x1 + acc 2x1 + pvacc 2x1 + wide 1x2
        pst = ctx.enter_context(tc.tile_pool(name="ps_t", bufs=2, space="PSUM"))
        psa = ctx.enter_context(tc.tile_pool(name="ps_a", bufs=2, space="PSUM"))
        psv = ctx.enter_context(tc.tile_pool(name="ps_v", bufs=2, space="PSUM"))
        psw = ctx.enter_context(tc.tile_pool(name="ps_w", bufs=1, space="PSUM"))
        dr = ctx.enter_context(tc.tile_pool(name="dram", bufs=1, space="DRAM"))

        cp_engs = [nc.vector, nc.gpsimd]

        def cpe():
            return nc.vector

        idn = pp.tile([P, P], DT, tag="idn", name="idn")
        nc.sync.dma_start(out=idn[:], in_=I["idn"][:])
        idn32 = pp.tile([P, P], F32, tag="idn32", name="idn32")
        nc.sync.dma_start(out=idn32[:], in_=I["idn32"][:])

        gB = pp.tile([P, D], F32, tag="gB", name="gB")
        bB = pp.tile([P, D], F32, tag="bB", name="bB")

        def load_gb(gname, bname):
            brow = sp.tile([1, D], F32, tag="brow", name="brow", bufs=1)
            nc.sync.dma_start(out=brow[:], in_=I[gname][:])
            nc.gpsimd.partition_broadcast(gB[:], brow[:])
            brow2 = sp.tile([1, D], F32, tag="brow2", name="brow2", bufs=1)
            nc.sync.dma_start(out=brow2[:], in_=I[bname][:])
            nc.gpsimd.partition_broadcast(bB[:], brow2[:])

        load_gb("g1", "b1")
        bmS = pp.tile([P, 32], DT, tag="bmS", name="bmS")
        nc.sync.dma_start(out=bmS[:], in_=I["bmS"][:])
        A4 = pp.tile([NEXP, P], F32, tag="A4", name="A4")
        nc.sync.dma_start(out=A4[:], in_=I["A4"][:])
        B16 = pp.tile([NEXP, 16], F32, tag="B16", name="B16")
        nc.sync.dma_start(out=B16[:], in_=I["B16"][:])

        def group_cols(wcol_ap, ncols):
            """wcol [64, ncols] f32 -> [128, 16*ncols] where
            out[k, g*ncols+p] = wcol[4g + k%4, p]."""
            rhsB = sp.tile([NEXP, 16 * ncols], F32, tag="rhsB", name="rhsB")
            for pi in range(ncols):
                nc.vector.tensor_scalar(
                    out=rhsB[:, pi:16 * ncols:ncols], in0=B16[:],
                    scalar1=wcol_ap[:, pi:pi + 1], scalar2=None, op0=ALU.mult)
            wkp = pst.tile([P, 16 * ncols], F32, tag="tpp", name="wkp")
            nc.tensor.matmul(out=wkp[:], lhsT=A4[:], rhs=rhsB[:],
                             start=True, stop=True)
            wk = sp.tile([P, 16 * ncols], F32, tag="wkall", name="wkall")
            nc.vector.tensor_copy(out=wk[:], in_=wkp[:])
            return wk

        def layernorm_tile(x_ap, pool, tag):
            """LN over free dim (D) of [P, D] fp32 -> [P, D] DT tile."""
            stats = sp.tile([P, 4], F32, tag="ln_stats", name="ln_stats")
            mean = stats[:, 0:1]; var = stats[:, 1:2]; rstd = stats[:, 2:3]
            nmean = stats[:, 3:4]
            nc.vector.tensor_reduce(out=mean, in_=x_ap, axis=AX, op=ALU.add)
            nc.vector.tensor_scalar(out=nmean, in0=mean, scalar1=-1.0 / D,
                                    scalar2=None, op0=ALU.mult)
            cent = sp.tile([P, D], F32, tag="ln_cent", name="ln_cent", bufs=1)
            nc.scalar.activation(out=cent[:], in_=x_ap, func=ACT.Identity, bias=nmean)
            sq = sp.tile([P, D], F32, tag="ln_sq", name="ln_sq", bufs=1)
            nc.scalar.activation(out=sq[:], in_=cent[:], func=ACT.Square,
                                 accum_out=var)
            nc.vector.tensor_scalar(out=var, in0=var, scalar1=1.0 / D,
                                    scalar2=1e-5, op0=ALU.mult, op1=ALU.add)
            nc.scalar.sqrt(var, var)
            nc.vector.reciprocal(rstd, var)
            out = pool.tile([P, D], DT, tag=tag)
            nc.vector.scalar_tensor_tensor(
                out=out[:], in0=cent[:], scalar=rstd,
                in1=gB[:], op0=ALU.mult, op1=ALU.mult)
            nc.gpsimd.tensor_tensor(out=out[:], in0=out[:],
                                    in1=bB[:], op=ALU.add)
            return out

        def softmax_blocks(psum_ap, out_ap, nblk, blk):
            for bi in range(nblk):
                sl = slice(bi * blk, (bi + 1) * blk)
                st = sp.tile([P, 2], F32, tag="sm_st", name="sm_st")
                mx = st[:, 0:1]; se = st[:, 1:2]
                nc.vector.tensor_reduce(out=mx, in_=psum_ap[:, sl], axis=AX,
                                        op=ALU.max)
                nc.vector.tensor_scalar(out=mx, in0=mx, scalar1=-1.0,
                                        scalar2=None, op0=ALU.mult)
                nc.scalar.activation(out=out_ap[:, sl], in_=psum_ap[:, sl],
                                     func=ACT.Exp, bias=mx, accum_out=se)
                nc.vector.reciprocal(se, se)
                nc.vector.tensor_scalar(out=out_ap[:, sl], in0=out_ap[:, sl],
                                        scalar1=se, scalar2=None, op0=ALU.mult)

        # ---- load x; layernorm -> nx (DT); transpose -> nxT ----
        xt = [pp.tile([P, D], F32, tag=f"x{i}", name=f"x{i}") for i in range(QT)]
        with tc.tile_pool(name="ph0", bufs=1) as p0:
            nxT = [p0.tile([P, S], DT, tag=f"nxT{t}", name=f"nxT{t}") for t in range(DT_T)]
            for i in range(NT if "ln0" not in ABLATE else 0):
                if i < QT:
                    xi = xt[i]
                else:
                    xi = sp.tile([P, D], F32, tag="x_hi", name="x_hi")
                nc.sync.dma_start(out=xi[:], in_=I["x"][i * P:(i + 1) * P, :])
                nx_i = layernorm_tile(xi[:], sp, "nx")
                for t in range(DT_T):
                    tp = pst.tile([P, P], DT, tag="tpp", name="tpp")
                    nc.tensor.transpose(out=tp[:], in_=nx_i[:, t * P:(t + 1) * P],
                                        identity=idn[:])
                    cpe().tensor_copy(out=nxT[t][:, i * P:(i + 1) * P], in_=tp[:])

            # ---- routers (c,q,k,v) ----
            wctt = [p0.tile([P, 4 * NEXP], DT, tag=f"wct{t}", name=f"wct{t}") for t in range(DT_T)]
            for t in range(DT_T):
                nc.sync.dma_start(out=wctt[t][:], in_=I["wct"][t * P:(t + 1) * P, :])
            imp_t = [pp.tile([P, 1], DT, tag=f"imp{i}", name=f"imp{i}") for i in range(NT)]
            for i in range(NT):
                nc.sync.dma_start(out=imp_t[i][:], in_=I["imp"][i * P:(i + 1) * P, :])

            wpool_ps = psv.tile([1, 4 * NEXP], F32, tag="pvacc", name="pvacc")
            for i in range(NT):
                pr_ps = psa.tile([P, 4 * NEXP], F32, tag="acc", name="acc")
                for t in range(DT_T):
                    nc.tensor.matmul(out=pr_ps[:],
                                     lhsT=nxT[t][:, i * P:(i + 1) * P],
                                     rhs=wctt[t][:], start=(t == 0),
                                     stop=(t == DT_T - 1))
                pref = sp.tile([P, 4 * NEXP], DT, tag="pref", name="pref")
                softmax_blocks(pr_ps[:], pref[:], 4, NEXP)
                nc.tensor.matmul(out=wpool_ps[:], lhsT=imp_t[i][:], rhs=pref[:],
                                 start=(i == 0), stop=(i == NT - 1))

            wrow = pp.tile([1, 4 * NEXP], F32, tag="wrow", name="wrow")
            nc.vector.tensor_copy(out=wrow[:], in_=wpool_ps[:])
            for bi in range(4):
                sl = slice(bi * NEXP, (bi + 1) * NEXP)
                st = sp.tile([1, 1], F32, tag="wn_st", name="wn_st")
                nc.vector.tensor_reduce(out=st[:], in_=wrow[:, sl], axis=AX,
                                        op=ALU.add)
                nc.vector.tensor_scalar(out=st[:], in0=st[:], scalar1=1e-8,
                                        scalar2=None, op0=ALU.add)
                nc.vector.reciprocal(st[:], st[:])
                nc.vector.tensor_scalar(out=wrow[:, sl], in0=wrow[:, sl],
                                        scalar1=st[:], scalar2=None, op0=ALU.mult)
            # w columns (fp32): wcolcq = [cw|qw], wcolkv = [kw|vw]
            wt0 = pst.tile([P, 1], F32, tag="tpp", name="wt0")
            nc.tensor.transpose(out=wt0[:], in_=wrow[:, 0:P],
                                identity=idn32[0:1, 0:1])
            wt1 = pst.tile([P, 1], F32, tag="tpp", name="wt1")
            nc.tensor.transpose(out=wt1[:], in_=wrow[:, P:2 * P],
                                identity=idn32[0:1, 0:1])
            wcolcq = pp.tile([P, 1], F32, tag="wcolcq", name="wcolcq")
            nc.vector.tensor_copy(out=wcolcq[:], in_=wt0[:])
            wcolkv = pp.tile([P, 1], F32, tag="wcolkv", name="wcolkv")
            nc.vector.tensor_copy(out=wcolkv[:], in_=wt1[:])
            # wcols3 [n, (q,k,v)]; rows 64+ zero. wcolsZ: only row 63 kept.
            wcols3 = pp.tile([P, 3], F32, tag="wcols3", name="wcols3")
            nc.vector.memset(wcols3[:], 0.0)
            nc.vector.tensor_copy(out=wcols3[0:NEXP, 0:1], in_=wcolcq[NEXP:2 * NEXP, :])
            nc.vector.tensor_copy(out=wcols3[0:NEXP, 1:2], in_=wcolkv[0:NEXP, :])
            nc.vector.tensor_copy(out=wcols3[0:NEXP, 2:3], in_=wcolkv[NEXP:2 * NEXP, :])

            if DBG:
                nc.sync.dma_start(out=DBG["o_wr"][:], in_=wrow[:])
            # ---- sc = sum_n cw[n]*CN[n] -> [P(d%128), (dtile, r)] ----
            # stacked-K: K = 32 chunks x 4 experts/group, M = 32 chunks.
            def combine_cn(wcol, out_tag, lpool):
                wkall = group_cols(wcol, 1)
                lhs = []
                for g in range(16):
                    lg = lpool.tile([P, 32], DT, tag=f"sclh{g}", name=f"sclh{g}")
                    nc.vector.tensor_scalar(out=lg[:], in0=bmS[:],
                                            scalar1=wkall[:, g:g + 1],
                                            scalar2=None, op0=ALU.mult)
                    lhs.append(lg)
                out = pp.tile([P, D], DT, tag=out_tag)
                FLC = D * R
                for s in range(8 if "comb" not in ABLATE else 0):
                    acc = psa.tile([32, 512], F32, tag="acc", name="cacc")
                    for g in range(16):
                        rhs = sp.tile([P, 512], DT, tag="c_rhs", name="c_rhs")
                        srcap = bass.AP(I["cn"].tensor, 4 * g * FLC + 32 * s * 512,
                                        [[512, 32], [FLC, 4], [1, 512]])
                        nc.sync.dma_start(out=rhs[:], in_=srcap)
                        nc.tensor.matmul(out=acc[:], lhsT=lhs[g][:], rhs=rhs[:],
                                         start=(g == 0), stop=(g == 15))
                    pks = sp.tile([32, 512], DT, tag="c_pks", name="c_pks")
                    cpe().tensor_copy(out=pks[:], in_=acc[:])
                    # (c,fh,fr): dst partition 4c+fh, free s*128+fr
                    nc.sync.dma_start(out=out[:, s * P:(s + 1) * P],
                                      in_=pks[:])
                return out

            sc_b = combine_cn(wcolcq[0:NEXP, 0:1], "sc_b", p0)
            if DBG:
                nc.sync.dma_start(out=DBG["o_sc"][:], in_=sc_b[:])

            # ---- e3 = (eQ|eK|eV) [r, d]: stacked-K (32 chunks x 4 experts) ----
            e3 = pp.tile([P, 3 * D], DT, tag="e3", name="e3")
            FL = R * D
            w3all = group_cols(wcols3[0:NEXP, :], 3)  # [128, 48]
            lhs3 = []
            for g in range(16):
                lg = p0.tile([P, 96], DT, tag=f"e3lh{g}", name=f"e3lh{g}")
                for pl_i in range(3):
                    nc.vector.tensor_scalar(
                        out=lg[:, pl_i:96:3], in0=bmS[:],
                        scalar1=w3all[:, 3 * g + pl_i:3 * g + pl_i + 1],
                        scalar2=None, op0=ALU.mult)
                lhs3.append(lg)
            for s in range(8 if "e3" not in ABLATE else 0):
                acc = psa.tile([96, 512], F32, tag="acc", name="eacc")
                for g in range(16):
                    rhs = sp.tile([P, 512], DT, tag="e_rhs", name="e_rhs")
                    srcap = bass.AP(I["pl"].tensor, 4 * g * FL + 32 * s * 512,
                                    [[512, 32], [FL, 4], [1, 512]])
                    nc.sync.dma_start(out=rhs[:], in_=srcap)
                    nc.tensor.matmul(out=acc[:], lhsT=lhs3[g][:], rhs=rhs[:],
                                     start=(g == 0), stop=(g == 15))
                pks = sp.tile([96, 512], DT, tag="e_pks", name="e_pks")
                cpe().tensor_copy(out=pks[:], in_=acc[:])
                # bounce via dram: row m = c*3+pl (c = 2*c2+ch)
                e3d = dr.tile([96, 512], DT, tag="e3d", name="e3d")
                nc.sync.dma_start(out=e3d[:], in_=pks[:])
                for pl_i in range(3):
                    for ch in range(2):
                        srcd = bass.AP(e3d.tensor, (3 * ch + pl_i) * 512,
                                       [[6 * 512, 16], [1, 512]])
                        nc.sync.dma_start(
                            out=e3[16 * s:16 * s + 16,
                                   pl_i * D + ch * 512:pl_i * D + ch * 512 + 512],
                            in_=srcd)
            # ---- h -> hT; then KT, QT, Vext ----
            hT = pp.tile([P, S], DT, tag="hT")
            for i in range(NT):
                hp = psa.tile([P, R], F32, tag="acc", name="acc")
                for t in range(DT_T):
                    nc.tensor.matmul(out=hp[:],
                                     lhsT=nxT[t][:, i * P:(i + 1) * P],
                                     rhs=sc_b[:, t * P:(t + 1) * P],
                                     start=(t == 0), stop=(t == DT_T - 1))
                hb = sp.tile([P, R], DT, tag="h_b", name="h_b")
                nc.vector.tensor_copy(out=hb[:], in_=hp[:])
                htp = pst.tile([P, P], DT, tag="tpp", name="tpp")
                nc.tensor.transpose(out=htp[:], in_=hb[:], identity=idn[:])
                cpe().tensor_copy(out=hT[:, i * P:(i + 1) * P], in_=htp[:])

            if DBG:
                nc.sync.dma_start(out=DBG["o_hT"][:], in_=hT[:])
            SCALE_Q = 1.0 / float(np.sqrt(DH))
            kT = [pp.tile([P, S], DT, tag=f"kT{t}", name=f"kT{t}") for t in range(DT_T)]
            qT = [pp.tile([P, SQ], DT, tag=f"qT{t}", name=f"qT{t}") for t in range(DT_T)]
            vext = [pp.tile([P, H * (DH + 1)], DT, tag=f"vx{i}", name=f"vx{i}") for i in range(NT)]
            for t in range(DT_T):
                kp = psw.tile([P, S], F32, tag="wide", name="wide")
                for j in range(2):
                    nc.tensor.matmul(out=kp[:, j * 512:(j + 1) * 512],
                                     lhsT=e3[:, D + t * P:D + t * P + P],
                                     rhs=hT[:, j * 512:(j + 1) * 512],
                                     start=True, stop=True)
                cpe().tensor_copy(out=kT[t][:], in_=kp[:])
                qp = psa.tile([P, SQ], F32, tag="acc", name="acc")
                nc.tensor.matmul(out=qp[:], lhsT=e3[:, t * P:t * P + P],
                                 rhs=hT[:, 0:SQ], start=True, stop=True)
                nc.vector.tensor_scalar(out=qT[t][:], in0=qp[:], scalar1=SCALE_Q,
                                        scalar2=None, op0=ALU.mult)
            for i in range(NT):
                vp = psw.tile([P, D], F32, tag="wide", name="wide")
                for j in range(2):
                    nc.tensor.matmul(
                        out=vp[:, j * 512:(j + 1) * 512],
                        lhsT=hT[:, i * P:(i + 1) * P],
                        rhs=e3[:, 2 * D + j * 512:2 * D + (j + 1) * 512],
                        start=True, stop=True)
                vv = vext[i][:].rearrange("p (hh c) -> p hh c", c=DH + 1)
                cpe().tensor_copy(out=vv[:, :, 0:DH],
                                  in_=vp[:].rearrange("p (hh c) -> p hh c", c=DH))
                nc.vector.memset(vv[:, :, DH:DH + 1], 1.0)
        # ph0 pool (nxT, wctt) released here

        # ---- additive mask tiles [key-tile, SQ] ----
        with tc.tile_pool(name="ph4", bufs=1) as p4:
            madd = [p4.tile([P, SQ], DT, tag=f"madd{i}", name=f"madd{i}") for i in range(NT)]
            for i in range(NT):
                mu = sp.tile([P, SQ], U8, tag="mu8", name="mu8")
                nc.sync.dma_start(out=mu[:], in_=I["mT"][i * P:(i + 1) * P, :])
                nc.vector.tensor_scalar(out=madd[i][:], in0=mu[:],
                                        scalar1=float(-NEG), scalar2=float(NEG),
                                        op0=ALU.mult, op1=ALU.add)

            # ---- attention ----
            attnT = [pp.tile([P, SQ], DT, tag=f"at{t}", name=f"at{t}") for t in range(DT_T)]
            for hd in range(H if "attn" not in ABLATE else 0):
                t4 = hd // 2
                hs = (hd % 2) * DH
                po = psv.tile([DH + 1, SQ], F32, tag="pvacc", name="pvacc")
                for kb in range(NT):
                    sps = psa.tile([P, SQ], F32, tag="acc", name="acc")
                    nc.tensor.matmul(out=sps[:],
                                     lhsT=kT[t4][hs:hs + DH, kb * P:(kb + 1) * P],
                                     rhs=qT[t4][hs:hs + DH, :],
                                     start=True, stop=False)
                    nc.tensor.matmul(out=sps[:], lhsT=idn[:], rhs=madd[kb][:],
                                     start=False, stop=True)
                    pt = sp.tile([P, SQ], DT, tag="p_tile", name="p_tile")
                    nc.scalar.activation(out=pt[:], in_=sps[:], func=ACT.Exp)
                    nc.tensor.matmul(
                        out=po[:],
                        lhsT=vext[kb][:, hd * (DH + 1):(hd + 1) * (DH + 1)],
                        rhs=pt[:], start=(kb == 0), stop=(kb == NT - 1))
                rec = sp.tile([1, SQ], F32, tag="rec", name="rec", bufs=1)
                nc.vector.reciprocal(rec[:], po[DH:DH + 1, :])
                recB = sp.tile([DH, SQ], F32, tag="recB", name="recB")
                nc.gpsimd.partition_broadcast(recB[:], rec[:])
                nc.vector.tensor_tensor(out=attnT[t4][hs:hs + DH, :],
                                        in0=po[0:DH, :],
                                        in1=recB[:], op=ALU.mult)

            # ---- WO + residual -> y ----
            wot_t = [p4.tile([P, D], DT, tag=f"wot{t}", name=f"wot{t}") for t in range(DT_T)]
            for t in range(DT_T):
                nc.sync.dma_start(out=wot_t[t][:], in_=I["wot"][t * P:(t + 1) * P, :])
            yt = [pp.tile([P, D], F32, tag=f"y{i}", name=f"y{i}") for i in range(QT)]
            for i in range(QT):
                wp = psw.tile([P, D], F32, tag="wide", name="wide")
                for j in range(2):
                    for t in range(DT_T):
                        nc.tensor.matmul(out=wp[:, j * 512:(j + 1) * 512],
                                         lhsT=attnT[t][:, i * P:(i + 1) * P],
                                         rhs=wot_t[t][:, j * 512:(j + 1) * 512],
                                         start=(t == 0), stop=(t == DT_T - 1))
                nc.vector.tensor_tensor(out=yt[i][:], in0=wp[:], in1=xt[i][:],
                                        op=ALU.add)
            if DBG:
                nc.sync.dma_start(out=DBG["o_at"][:], in_=attnT[0][:])
                for i in range(QT):
                    nc.sync.dma_start(out=DBG["o_y"][i * P:(i + 1) * P, :],
                                      in_=yt[i][:])
        # ph4 released

        # ---- memory block ----
        with tc.tile_pool(name="ph6", bufs=1) as p6:
            nx2T = [p6.tile([P, SQ], DT, tag=f"n2T{t}", name=f"n2T{t}") for t in range(DT_T)]
            load_gb("g2", "b2")
            for i in range(QT):
                nx2_i = layernorm_tile(yt[i][:], sp, "nx2")
                for t in range(DT_T):
                    tp = pst.tile([P, P], DT, tag="tpp", name="tpp")
                    nc.tensor.transpose(out=tp[:], in_=nx2_i[:, t * P:(t + 1) * P],
                                        identity=idn[:])
                    cpe().tensor_copy(out=nx2T[t][:, i * P:(i + 1) * P], in_=tp[:])

            wmtt = [p6.tile([P, NEXP], DT, tag=f"wmt{t}", name=f"wmt{t}") for t in range(DT_T)]
            for t in range(DT_T):
                nc.sync.dma_start(out=wmtt[t][:], in_=I["wmt"][t * P:(t + 1) * P, :])
            mwp_ps = psv.tile([1, NEXP], F32, tag="pvacc", name="pvacc")
            for i in range(QT):
                pr = psa.tile([P, NEXP], F32, tag="acc", name="acc")
                for t in range(DT_T):
                    nc.tensor.matmul(out=pr[:],
                                     lhsT=nx2T[t][:, i * P:(i + 1) * P],
                                     rhs=wmtt[t][:], start=(t == 0),
                                     stop=(t == DT_T - 1))
                prefm = sp.tile([P, NEXP], DT, tag="prefm", name="prefm")
                softmax_blocks(pr[:], prefm[:], 1, NEXP)
                nc.tensor.matmul(out=mwp_ps[:], lhsT=imp_t[i][:], rhs=prefm[:],
                                 start=(i == 0), stop=(i == QT - 1))

            bselr = p6.tile([1, B], F32, tag="bselr", name="bselr")
            nc.sync.dma_start(out=bselr[:], in_=I["bselr"][:])
            bselc = p6.tile([B, 1], F32, tag="bselc", name="bselc")
            nc.sync.dma_start(out=bselc[:], in_=I["bselc"][:])
            mwrow = p6.tile([1, NEXP], F32, tag="mwrow", name="mwrow")
            if use_cc:
                mwr = sp.tile([1, NEXP], F32, tag="mwr", name="mwr")
                nc.vector.tensor_copy(out=mwr[:], in_=mwp_ps[:])
                ccp = psa.tile([B, NEXP], F32, tag="acc", name="acc")
                nc.tensor.matmul(out=ccp[:], lhsT=bselr[:], rhs=mwr[:],
                                 start=True, stop=True)
                cc_sb = sp.tile([B, NEXP], F32, tag="cc_sb", name="cc_sb")
                nc.vector.tensor_copy(out=cc_sb[:], in_=ccp[:])
                cc_in = dr.tile([B, NEXP], F32)
                cc_out = dr.tile([B, NEXP], F32)
                nc.gpsimd.dma_start(out=cc_in[:], in_=cc_sb[:])
                nc.gpsimd.collective_compute(
                    "AllReduce", ALU.add,
                    replica_groups=[list(range(N_CORES))],
                    ins=[cc_in.opt()], outs=[cc_out.opt()])
                cc_res = sp.tile([B, NEXP], F32, tag="cc_res", name="cc_res")
                nc.gpsimd.dma_start(out=cc_res[:], in_=cc_out[:])
                mwf = psa.tile([1, NEXP], F32, tag="acc", name="acc")
                nc.tensor.matmul(out=mwf[:], lhsT=bselc[:], rhs=cc_res[:],
                                 start=True, stop=True)
                nc.vector.tensor_copy(out=mwrow[:], in_=mwf[:])
            else:
                nc.vector.tensor_copy(out=mwrow[:], in_=mwp_ps[:])
            st = sp.tile([1, 1], F32, tag="wn_st", name="wn_st")
            nc.vector.tensor_reduce(out=st[:], in_=mwrow[:], axis=AX, op=ALU.add)
            nc.vector.tensor_scalar(out=st[:], in0=st[:], scalar1=1e-8,
                                    scalar2=None, op0=ALU.add)
            nc.vector.reciprocal(st[:], st[:])
            nc.vector.tensor_scalar(out=mwrow[:], in0=mwrow[:], scalar1=st[:],
                                    scalar2=None, op0=ALU.mult)

            mwrow_cp = sp.tile([1, NEXP], F32, tag="mwr2", name="mwr2")
            nc.vector.tensor_copy(out=mwrow_cp[:], in_=mwrow[:])
            mwt = pst.tile([NEXP, 1], F32, tag="tpp", name="mwt")
            nc.tensor.transpose(out=mwt[:], in_=mwrow_cp[:],
                                identity=idn32[0:1, 0:1])
            mwcol = p6.tile([NEXP, 1], F32, tag="mwcol", name="mwcol")
            nc.vector.tensor_copy(out=mwcol[:], in_=mwt[:])
            if DBG:
                nc.sync.dma_start(out=DBG["o_mw"][:], in_=mwrow[:])
            scm_b = combine_cn(mwcol[0:NEXP, 0:1], "scm_b", p6)

            # QmT [r, sq], 1/sqrt(KR) folded
            qmp = psa.tile([P, SQ], F32, tag="acc", name="acc")
            for t in range(DT_T):
                nc.tensor.matmul(out=qmp[:], lhsT=scm_b[:, t * P:(t + 1) * P],
                                 rhs=nx2T[t][:], start=(t == 0),
                                 stop=(t == DT_T - 1))
            qmT = p6.tile([P, SQ], DT, tag="qmT")
            nc.vector.tensor_scalar(out=qmT[:], in0=qmp[:],
                                    scalar1=1.0 / float(np.sqrt(KR)),
                                    scalar2=None, op0=ALU.mult)

            kkt_sb = p6.tile([P, NK], DT, tag="kkt", name="kkt")
            nc.sync.dma_start(out=kkt_sb[:], in_=I["kkt"][:])

            idx_all = p6.tile([P, QT * TOPK], U32, tag="idx_all", name="idx_all")
            w8_all = p6.tile([P, QT * TOPK], F32, tag="w8_all", name="w8_all")
            for i in range(QT if "ks" not in ABLATE else 0):
                ks = sp.tile([P, NK], F32, tag="ks_sb", name="ks_sb", bufs=1)
                for j in range(NK // 512):
                    ksp = psa.tile([P, 512], F32, tag="acc", name="acc")
                    nc.tensor.matmul(out=ksp[:], lhsT=qmT[:, i * P:(i + 1) * P],
                                     rhs=kkt_sb[:, j * 512:(j + 1) * 512],
                                     start=True, stop=True)
                    cpe().tensor_copy(out=ks[:, j * 512:(j + 1) * 512], in_=ksp[:])
                if DBG and i == 0:
                    nc.sync.dma_start(out=DBG["o_ks"][:], in_=ks[:])
                tv = sp.tile([P, TOPK], F32, tag="tv", name="tv")
                nc.vector.max(out=tv[:], in_=ks[:])
                nc.vector.max_index(out=idx_all[:, i * TOPK:(i + 1) * TOPK],
                                    in_max=tv[:], in_values=ks[:])
                if DBG and i == 0:
                    nc.sync.dma_start(out=DBG["o_idx"][:],
                                      in_=idx_all[:, 0:TOPK])
                st8 = sp.tile([P, 2], F32, tag="st8", name="st8")
                nm = st8[:, 0:1]; se8 = st8[:, 1:2]
                nc.vector.tensor_scalar(out=nm, in0=tv[:, 0:1], scalar1=-1.0,
                                        scalar2=None, op0=ALU.mult)
                w8 = sp.tile([P, TOPK], F32, tag="w8", name="w8")
                nc.scalar.activation(out=w8[:], in_=tv[:], func=ACT.Exp,
                                     bias=nm, accum_out=se8)
                nc.vector.reciprocal(se8, se8)
                nc.vector.tensor_scalar(out=w8_all[:, i * TOPK:(i + 1) * TOPK],
                                        in0=w8[:], scalar1=se8, scalar2=None,
                                        op0=ALU.mult)

            for i in range(QT if "gath" not in ABLATE else 0):
                acc = sp.tile([P, D], F32, tag="mem_acc", name="mem_acc", bufs=1)
                for k in range(TOPK):
                    g = i * TOPK + k
                    gt = sp.tile([P, D], DT, tag="gath", name="gath")
                    nc.gpsimd.indirect_dma_start(
                        out=gt[:], out_offset=None, in_=I["kv"][:],
                        in_offset=bass.IndirectOffsetOnAxis(
                            ap=idx_all[:, g:g + 1], axis=0))
                    prev = yt[i][:] if k == 0 else acc[:]
                    nc.vector.scalar_tensor_tensor(
                        out=acc[:], in0=gt[:], scalar=w8_all[:, g:g + 1],
                        in1=prev, op0=ALU.mult, op1=ALU.add)
                nc.sync.dma_start(out=o[i * P:(i + 1) * P, :], in_=acc[:])


# ---------------- PJRT SPMD runner (persistent jit) ----------------

class SpmdRunner:
    def __init__(self, nc, n_cores):
        import jax
        from jax.sharding import Mesh, PartitionSpec
        from jax.experimental.shard_map import shard_map
        from concourse import bass2jax
        bass2jax.install_neuronx_cc_hook()
        self.jax = jax
        self.nc = nc
        self.n_cores = n_cores
        partition_name = (nc.partition_id_tensor.name
                          if nc.partition_id_tensor else None)
        in_names, out_names, out_avals, zero_outs = [], [], [], []
        for alloc in nc.m.functions[0].allocations:
            if not isinstance(alloc, mybir.MemoryLocationSet):
                continue
            name = alloc.memorylocations[0].name
            if alloc.kind == "ExternalInput":
                if name != partition_name:
                    in_names.append(name)
            elif alloc.kind == "ExternalOutput":
                shape = tuple(alloc.tensor_shape)
                dtype = mybir.dt.np(alloc.dtype)
                out_names.append(name)
                out_avals.append(jax.core.ShapedArray(shape, dtype))
                zero_outs.append(np.zeros(shape, dtype))
        self.n_params = len(in_names)
        self.in_names = list(in_names)
        self.out_names = out_names
        self.out_avals = out_avals
        self.zero_outs = zero_outs
        all_in = in_names + out_names + ([partition_name] if partition_name else [])

        def _body(*args):
            operands = list(args)
            if partition_name is not None:
                operands.append(bass2jax.partition_id_tensor())
            outs = bass2jax._bass_exec_p.bind(
                *operands, out_avals=tuple(out_avals), in_names=tuple(all_in),
                out_names=tuple(out_names), lowering_input_output_aliases=(),
                sim_require_finite=True, sim_require_nnan=True, nc=nc)
            return tuple(outs)

        devices = jax.devices()[:n_cores]
        self.mesh = Mesh(np.asarray(devices), ("core",))
        nspec = self.n_params + len(out_names)
        self.sharded = jax.jit(
            shard_map(_body, mesh=self.mesh,
                      in_specs=(PartitionSpec("core"),) * nspec,
                      out_specs=(PartitionSpec("core"),) * len(out_names),
                      check_rep=False),
            keep_unused=True)

    def concat_inputs(self, in_maps):
        per_core = [[np.asarray(m[n]) for n in self.in_names] for m in in_maps]
        cat = [np.concatenate([per_core[c][i] for c in range(self.n_cores)],
                              axis=0) for i in range(self.n_params)]
        cat += [np.zeros((self.n_cores * z.shape[0], *z.shape[1:]), z.dtype)
                for z in self.zero_outs]
        return cat

    def run(self, in_maps):
        out_arrs = self.sharded(*self.concat_inputs(in_maps))
        self.jax.block_until_ready(out_arrs)
        return [
            {n: np.asarray(out_arrs[i]).reshape(
                self.n_cores, *self.out_avals[i].shape)[c]
             for i, n in enumerate(self.out_names)}
            for c in range(self.n_cores)
        ]


# ---------------- host side ----------------

_RUNNER = None


NEXP_ = NEXP


def _make_inputs(x, importance, mask, compress_neurons, expand_pool,
                 knowledge_K, knowledge_V, Wc, WQ, WK, WV, Wm, WO,
                 g1, b1, g2, b2):
    ndt = np_bdt()
    f = lambda a: np.asarray(a, np.float32)
    cn = f(compress_neurons).astype(ndt)
    pl = f(expand_pool).astype(ndt)
    wct = np.ascontiguousarray(
        np.concatenate([f(Wc), f(WQ), f(WK), f(WV)], axis=0).T).astype(ndt)
    wmt = np.ascontiguousarray(f(Wm).T).astype(ndt)
    wot = np.ascontiguousarray(f(WO).T).astype(ndt)
    kkt = np.ascontiguousarray(f(knowledge_K).T).astype(ndt)
    kv = f(knowledge_V).astype(ndt)
    idn32 = np.eye(P, dtype=np.float32)
    idn = idn32.astype(ndt)
    bmS = ((np.arange(P)[:, None] // 4) == np.arange(32)[None, :]).astype(np.float32)
    A4 = ((np.arange(NEXP)[:, None] % 4) == (np.arange(P)[None, :] % 4)).astype(np.float32)
    B16 = ((np.arange(NEXP)[:, None] // 4) == np.arange(16)[None, :]).astype(np.float32)
    shared = dict(cn=cn, pl=pl, wct=wct, wmt=wmt, wot=wot, kkt=kkt, kv=kv,
                  g1=f(g1).reshape(1, D), b1=f(b1).reshape(1, D),
                  g2=f(g2).reshape(1, D), b2=f(b2).reshape(1, D),
                  idn=idn, idn32=idn32, bmS=bmS.astype(ndt), A4=A4, B16=B16)
    x = f(x); importance = f(importance); mask = np.asarray(mask)
    in_maps = []
    for c in range(N_CORES):
        b, hf = c // 2, c % 2
        qr = np.arange(hf * SQ, hf * SQ + SQ)
        rest = np.arange((1 - hf) * SQ, (1 - hf) * SQ + SQ)
        perm = np.concatenate([qr, rest])
        m = dict(shared)
        m["x"] = np.ascontiguousarray(x[b][perm])
        m["imp"] = importance[b][perm].reshape(S, 1).astype(ndt)
        mT = mask[b, 0][np.ix_(qr, perm)].T  # [key(perm), query(local)]
        m["mT"] = np.ascontiguousarray(mT).astype(np.uint8)
        onehot = np.zeros(B, np.float32); onehot[b] = 1.0
        m["bselr"] = onehot.reshape(1, B)
        m["bselc"] = onehot.reshape(B, 1)
        in_maps.append(m)
    return in_maps


def _get_runner():
    global _RUNNER
    if _RUNNER is None:
        nc = build_nc(use_cc=True)
        _RUNNER = SpmdRunner(nc, N_CORES)
    return _RUNNER


def kernel(**inputs):
    r = _get_runner()
    in_maps = _make_inputs(**inputs)
    res = r.run(in_maps)
    out = np.empty((B, S, D), np.float32)
    for c in range(N_CORES):
        b, hf = c // 2, c % 2
        out[b, hf * SQ:(hf + 1) * SQ] = res[c]["o"]
    return out



# revision 3
# speedup vs baseline: 2.0622x; 2.0622x over previous
"""DAWN block (moe_routing) Trainium2 kernel: 8-core SPMD, v2.

Sharding: core c = (batch b=c//2, half h=c%2). Each core handles one batch's
attention + memory block for half the queries (rows permuted so local queries
come first). Expert pools (compress_neurons / expand_pool) are pair-sharded:
each core streams only 32 of 64 experts and partial combines are AllReduced
within the pair. Causal structure: per-q-slot key-position lists + a constant
triangular mask tile + per-core bias column (full-mask blocks), so ~19% of
score/AV work is skipped and no per-key mask tensor is needed.

DMA strategy: everything is host-relaid-out so the device does few, large,
contiguous DMAs (the v1 kernel's 456 tiny combine DMAs were the bottleneck:
each DMA costs ~0.6us queue dispatch + 625ns shared HWDGE serial time).
"""
import numpy as np
import ml_dtypes

import concourse.bass as bass
import concourse.mybir as mybir
import concourse.tile as tile
from concourse import bacc

B, S, D = 4, 1024, 1024
H, DH = 16, 64
R = 128
NEXP = 64
NLOC = 32          # local experts per core (pair-sharded)
GLOC = NLOC // 4   # 8 stacked-expert groups
NK, KR = 4096, 128
TOPK = 8
N_CORES = 8
SQ = S // 2
P = 128
NT = S // P        # 8 seq tiles
QT = SQ // P       # 4 local q tiles
DT_T = D // P      # 8 d tiles

F32 = mybir.dt.float32
U32 = mybir.dt.uint32
NEG = -1.0e9
ALU = mybir.AluOpType
ACT = mybir.ActivationFunctionType
AX = None

# aux column maps
AF_A4 = 0          # [0:32, 0:128]
AF_B8 = 128        # [0:32, 128:136]
AF_BSELR = 136     # [0:1, 136:140]
AF_BSELC = 140     # [0:4, 140:141]
AF_NEGC = 141      # [0:128, 141:142]
AF_ONE = 142       # [0:1, 142:143]  value 1.0
AF_W = 144
AB_IDN = 0         # [0:128, 0:128]
AB_BMS = 128       # [0:128, 128:160]
AB_TRI = 160       # [0:128, 160:288]
AB_W = 288


def bdt():
    return mybir.dt.bfloat16


def np_bdt():
    return ml_dtypes.bfloat16


def build_nc(use_cc=True):
    global AX
    AX = mybir.AxisListType.X
    DT = bdt()
    nc = bacc.Bacc("TRN2", target_bir_lowering=False, debug=False,
                   num_devices=N_CORES)
    I = {}

    def inp(name, shape, dt):
        I[name] = nc.dram_tensor(name, shape, dt, kind="ExternalInput").ap()

    inp("x", [S, D], F32)              # row-permuted batch (local q first)
    inp("imp", [P, NT], DT)            # imp[p,i] = importance[perm[128i+p]]
    inp("cnb", [4, GLOC, P, 1024], DT) # local-expert compress pool, relaid
    inp("plb", [4, GLOC, P, 1024], DT) # local-expert expand pool, relaid
    inp("wct", [P, NT * 320], DT)      # [Wc|WQ|WK|WV|Wm].T tiled (expert-perm)
    inp("wot", [P, NT * 1024], DT)     # WO.T tiled
    inp("kkt", [KR, NK], DT)           # knowledge_K.T
    inp("kv", [NK, D], DT)             # knowledge_V
    inp("gb", [4, D], F32)             # g1,b1,g2,b2 rows
    inp("auxf", [P, AF_W], F32)
    inp("auxb", [P, AB_W], DT)
    o = nc.dram_tensor("o", [SQ, D], F32, kind="ExternalOutput").ap()

    with tile.TileContext(nc) as tc:
        _body(nc, tc, I, o, use_cc)
    nc.compile()
    return nc


def _body(nc, tc, I, o, use_cc):
    DT = bdt()
    import contextlib
    ctx = contextlib.ExitStack()
    with ctx:
        pp = ctx.enter_context(tc.tile_pool(name="pers", bufs=1))
        sp = ctx.enter_context(tc.tile_pool(name="stream", bufs=2))
        st2 = ctx.enter_context(tc.tile_pool(name="strm", bufs=2))
        pst = ctx.enter_context(tc.tile_pool(name="ps_t", bufs=2, space="PSUM"))
        psa = ctx.enter_context(tc.tile_pool(name="ps_a", bufs=2, space="PSUM"))
        psv = ctx.enter_context(tc.tile_pool(name="ps_v", bufs=2, space="PSUM"))
        dr = ctx.enter_context(tc.tile_pool(name="dram", bufs=1, space="DRAM"))

        # ---------- bulk loads ----------
        xa = pp.tile([P, NT * 1024], F32, tag="xa", name="xa")
        for hh in range(2):
            src = bass.AP(I["x"].tensor, hh * 4 * P * 1024,
                          [[1024, P], [P * 1024, 4], [1, 1024]])
            nc.sync.dma_start(out=xa[:, hh * 4096:(hh + 1) * 4096], in_=src)
        wct = pp.tile([P, NT * 320], DT, tag="wct", name="wct")
        nc.sync.dma_start(out=wct[:], in_=I["wct"][:])
        impa = pp.tile([P, NT], DT, tag="impa", name="impa")
        nc.sync.dma_start(out=impa[:], in_=I["imp"][:])
        auxf = pp.tile([P, AF_W], F32, tag="auxf", name="auxf")
        nc.sync.dma_start(out=auxf[:], in_=I["auxf"][:])
        auxb = pp.tile([P, AB_W], DT, tag="auxb", name="auxb")
        nc.sync.dma_start(out=auxb[:], in_=I["auxb"][:])
        gba = pp.tile([4, D], F32, tag="gba", name="gba")
        nc.sync.dma_start(out=gba[:], in_=I["gb"][:])

        idn = auxb[:, AB_IDN:AB_IDN + P]
        bmS = auxb[:, AB_BMS:AB_BMS + 32]
        tri = auxb[:, AB_TRI:AB_TRI + P]
        A4 = auxf[0:NLOC, AF_A4:AF_A4 + P]
        B8 = auxf[0:NLOC, AF_B8:AF_B8 + GLOC]
        bselr = auxf[0:1, AF_BSELR:AF_BSELR + B]
        bselc = auxf[0:B, AF_BSELC:AF_BSELC + 1]
        negc = auxf[:, AF_NEGC:AF_NEGC + 1]
        one1 = auxf[0:1, AF_ONE:AF_ONE + 1]

        # expert pool streams: cn (4), pl (4), wot (1), cn again (4)
        def stream_tile():
            return st2.tile([P, 8192], DT, tag="strm", name="strm")

        cn1 = []
        for b in range(4):
            t = stream_tile()
            src = bass.AP(I["cnb"].tensor, b * GLOC * P * 1024,
                          [[1024, P], [P * 1024, GLOC], [1, 1024]])
            nc.sync.dma_start(out=t[:], in_=src)
            cn1.append(t)
        pl1 = []
        for b in range(4):
            t = stream_tile()
            src = bass.AP(I["plb"].tensor, b * GLOC * P * 1024,
                          [[1024, P], [P * 1024, GLOC], [1, 1024]])
            nc.sync.dma_start(out=t[:], in_=src)
            pl1.append(t)
        wota = stream_tile()
        nc.sync.dma_start(out=wota[:], in_=I["wot"][:])
        cn2 = []
        for b in range(4):
            t = stream_tile()
            src = bass.AP(I["cnb"].tensor, b * GLOC * P * 1024,
                          [[1024, P], [P * 1024, GLOC], [1, 1024]])
            nc.sync.dma_start(out=t[:], in_=src)
            cn2.append(t)

        gB = pp.tile([P, D], F32, tag="gB", name="gB")
        bB = pp.tile([P, D], F32, tag="bB", name="bB")

        def load_gb(gi, bi):
            nc.gpsimd.partition_broadcast(gB[:], gba[gi:gi + 1, :])
            nc.gpsimd.partition_broadcast(bB[:], gba[bi:bi + 1, :])

        load_gb(0, 1)

        def layernorm_tile(x_ap, pool, tag):
            stats = sp.tile([P, 4], F32, tag="ln_stats", name="ln_stats")
            mean = stats[:, 0:1]; var = stats[:, 1:2]; rstd = stats[:, 2:3]
            nmean = stats[:, 3:4]
            nc.vector.tensor_reduce(out=mean, in_=x_ap, axis=AX, op=ALU.add)
            nc.vector.tensor_scalar(out=nmean, in0=mean, scalar1=-1.0 / D,
                                    scalar2=None, op0=ALU.mult)
            cent = sp.tile([P, D], F32, tag="ln_cent", name="ln_cent", bufs=1)
            nc.scalar.activation(out=cent[:], in_=x_ap, func=ACT.Identity,
                                 bias=nmean)
            sq = sp.tile([P, D], F32, tag="ln_sq", name="ln_sq", bufs=1)
            nc.scalar.activation(out=sq[:], in_=cent[:], func=ACT.Square,
                                 accum_out=var)
            nc.vector.tensor_scalar(out=var, in0=var, scalar1=1.0 / D,
                                    scalar2=1e-5, op0=ALU.mult, op1=ALU.add)
            nc.scalar.sqrt(var, var)
            nc.vector.reciprocal(rstd, var)
            out = pool.tile([P, D], DT, tag=tag)
            nc.vector.scalar_tensor_tensor(
                out=out[:], in0=cent[:], scalar=rstd,
                in1=gB[:], op0=ALU.mult, op1=ALU.mult)
            nc.gpsimd.tensor_tensor(out=out[:], in0=out[:], in1=bB[:],
                                    op=ALU.add)
            return out

        def softmax_blocks(psum_ap, out_ap, nblk, blk):
            for bi in range(nblk):
                sl = slice(bi * blk, (bi + 1) * blk)
                st = sp.tile([P, 2], F32, tag="sm_st", name="sm_st")
                mx = st[:, 0:1]; se = st[:, 1:2]
                nc.vector.tensor_reduce(out=mx, in_=psum_ap[:, sl], axis=AX,
                                        op=ALU.max)
                nc.vector.tensor_scalar(out=mx, in0=mx, scalar1=-1.0,
                                        scalar2=None, op0=ALU.mult)
                nc.scalar.activation(out=out_ap[:, sl], in_=psum_ap[:, sl],
                                     func=ACT.Exp, bias=mx, accum_out=se)
                nc.vector.reciprocal(se, se)
                nc.vector.tensor_scalar(out=out_ap[:, sl], in0=out_ap[:, sl],
                                        scalar1=se, scalar2=None, op0=ALU.mult)

        def group_cols(wcol_ap, ncols):
            """wcol [32, ncols] f32 -> wk [128, GLOC*ncols]:
            wk[p, ncols*g + c] = wcol[4g + p%4, c]."""
            rhsB = sp.tile([NLOC, GLOC * ncols], F32, tag="rhsB", name="rhsB")
            for pi in range(ncols):
                nc.vector.tensor_scalar(
                    out=rhsB[:, pi:GLOC * ncols:ncols], in0=B8,
                    scalar1=wcol_ap[:, pi:pi + 1], scalar2=None, op0=ALU.mult)
            wkp = pst.tile([P, GLOC * ncols], F32, tag="tpp", name="wkp")
            nc.tensor.matmul(out=wkp[:], lhsT=A4, rhs=rhsB[:],
                             start=True, stop=True)
            wk = sp.tile([P, GLOC * ncols], F32, tag="wkall", name="wkall")
            nc.vector.tensor_copy(out=wk[:], in_=wkp[:])
            return wk

        def combine_cn(wcol_ap, chunks, out_f32):
            """out_f32 [128, 1024] f32 partial combine of local experts.
            chunks[b][p, 1024g+128t+r] = CN[e(g,p), 128t+32b+p//4, r]."""
            wk = group_cols(wcol_ap, 1)
            lhs = []
            for g in range(GLOC):
                lg = sp.tile([P, NLOC], DT, tag=f"clh{g}", name=f"clh{g}",
                             bufs=1)
                nc.vector.tensor_scalar(out=lg[:], in0=bmS,
                                        scalar1=wk[:, g:g + 1],
                                        scalar2=None, op0=ALU.mult)
                lhs.append(lg)
            for b in range(4):
                acc = psa.tile([NLOC, 1024], F32, tag="acc", name="cacc")
                for hh in range(2):
                    for g in range(GLOC):
                        nc.tensor.matmul(
                            out=acc[:, hh * 512:(hh + 1) * 512],
                            lhsT=lhs[g][:],
                            rhs=chunks[b][:, g * 1024 + hh * 512:
                                          g * 1024 + (hh + 1) * 512],
                            start=(g == 0), stop=(g == GLOC - 1))
                nc.gpsimd.tensor_copy(out=out_f32[32 * b:32 * b + 32, :],
                                      in_=acc[:])

        def pair_allreduce(sb_f32, ncol):
            """AllReduce sb_f32 [128, ncol] within batch pairs (in place)."""
            if not use_cc:
                return
            cc_in = dr.tile([P, ncol], F32)
            cc_out = dr.tile([P, ncol], F32)
            nc.gpsimd.dma_start(out=cc_in[:], in_=sb_f32[:])
            nc.gpsimd.collective_compute(
                "AllReduce", ALU.add,
                replica_groups=[[0, 1], [2, 3], [4, 5], [6, 7]],
                ins=[cc_in.opt()], outs=[cc_out.opt()])
            nc.gpsimd.dma_start(out=sb_f32[:], in_=cc_out[:])

        # ---------- LN1 + transposes ----------
        ctx4 = contextlib.ExitStack()
        p4 = ctx4.enter_context(tc.tile_pool(name="ph4", bufs=1))
        with tc.tile_pool(name="ph0", bufs=1) as p0:
            nxT = [p0.tile([P, S], DT, tag=f"nxT{t}", name=f"nxT{t}")
                   for t in range(DT_T)]
            for i in range(NT):
                nx_i = layernorm_tile(xa[:, i * 1024:(i + 1) * 1024], sp, "nx")
                for t in range(DT_T):
                    tp = pst.tile([P, P], DT, tag="tpp", name="tpp")
                    nc.tensor.transpose(out=tp[:],
                                        in_=nx_i[:, t * P:(t + 1) * P],
                                        identity=idn)
                    eng = nc.gpsimd if (t % 2 == 0) else nc.vector
                    eng.tensor_copy(out=nxT[t][:, i * P:(i + 1) * P], in_=tp[:])

            # ---------- routers (c,q,k,v) ----------
            wpool_ps = psv.tile([1, 4 * NEXP], F32, tag="pvacc", name="pvacc")
            for i in range(NT):
                pr_ps = psa.tile([P, 4 * NEXP], F32, tag="acc", name="acc")
                for t in range(DT_T):
                    nc.tensor.matmul(out=pr_ps[:],
                                     lhsT=nxT[t][:, i * P:(i + 1) * P],
                                     rhs=wct[:, 320 * t:320 * t + 256],
                                     start=(t == 0), stop=(t == DT_T - 1))
                pref = sp.tile([P, 4 * NEXP], DT, tag="pref", name="pref")
                softmax_blocks(pr_ps[:], pref[:], 4, NEXP)
                nc.tensor.matmul(out=wpool_ps[:], lhsT=impa[:, i:i + 1],
                                 rhs=pref[:], start=(i == 0),
                                 stop=(i == NT - 1))

            wrow = pp.tile([1, 4 * NEXP], F32, tag="wrow", name="wrow")
            nc.vector.tensor_copy(out=wrow[:], in_=wpool_ps[:])
            for bi in range(4):
                sl = slice(bi * NEXP, (bi + 1) * NEXP)
                st = sp.tile([1, 1], F32, tag="wn_st", name="wn_st")
                nc.vector.tensor_reduce(out=st[:], in_=wrow[:, sl], axis=AX,
                                        op=ALU.add)
                nc.vector.tensor_scalar(out=st[:], in0=st[:], scalar1=1e-8,
                                        scalar2=None, op0=ALU.add)
                nc.vector.reciprocal(st[:], st[:])
                nc.vector.tensor_scalar(out=wrow[:, sl], in0=wrow[:, sl],
                                        scalar1=st[:], scalar2=None,
                                        op0=ALU.mult)
            wt0 = pst.tile([P, 1], F32, tag="tpp", name="wt0")
            nc.tensor.transpose(out=wt0[:], in_=wrow[:, 0:P], identity=one1)
            wt1 = pst.tile([P, 1], F32, tag="tpp", name="wt1")
            nc.tensor.transpose(out=wt1[:], in_=wrow[:, P:2 * P], identity=one1)
            wcolcq = pp.tile([P, 1], F32, tag="wcolcq", name="wcolcq")
            nc.vector.tensor_copy(out=wcolcq[:], in_=wt0[:])
            wcolkv = pp.tile([P, 1], F32, tag="wcolkv", name="wcolkv")
            nc.vector.tensor_copy(out=wcolkv[:], in_=wt1[:])
            wcols3 = pp.tile([NLOC, 3], F32, tag="wcols3", name="wcols3")
            nc.vector.tensor_copy(out=wcols3[:, 0:1],
                                  in_=wcolcq[NEXP:NEXP + NLOC, :])
            nc.vector.tensor_copy(out=wcols3[:, 1:2], in_=wcolkv[0:NLOC, :])
            nc.vector.tensor_copy(out=wcols3[:, 2:3],
                                  in_=wcolkv[NEXP:NEXP + NLOC, :])

            # ---------- sc combine (+pair AllReduce) ----------
            e3f = p0.tile([P, 3072], F32, tag="e3f", name="e3f")
            scf = e3f[:, 0:1024]
            combine_cn(wcolcq[0:NLOC, 0:1], cn1, scf)
            pair_allreduce(scf, 1024)
            sc_b = p0.tile([P, 1024], DT, tag="sc_b", name="sc_b")
            nc.vector.tensor_copy(out=sc_b[:], in_=scf[:])

            # ---------- e3 combine ----------
            w3 = group_cols(wcols3[:], 3)  # [128, 24]
            lhs3 = []
            for g in range(GLOC):
                lg = p0.tile([P, 96], DT, tag=f"e3lh{g}", name=f"e3lh{g}")
                for pl_i in range(3):
                    nc.vector.tensor_scalar(
                        out=lg[:, 32 * pl_i:32 * (pl_i + 1)], in0=bmS,
                        scalar1=w3[:, 3 * g + pl_i:3 * g + pl_i + 1],
                        scalar2=None, op0=ALU.mult)
                lhs3.append(lg)
            for b in range(4):
                acc = psa.tile([96, 1024], F32, tag="acc", name="eacc")
                for hh in range(2):
                    for g in range(GLOC):
                        nc.tensor.matmul(
                            out=acc[:, hh * 512:(hh + 1) * 512],
                            lhsT=lhs3[g][:],
                            rhs=pl1[b][:, g * 1024 + hh * 512:
                                       g * 1024 + (hh + 1) * 512],
                            start=(g == 0), stop=(g == GLOC - 1))
                for pl_i in range(3):
                    nc.gpsimd.tensor_copy(
                        out=e3f[32 * b:32 * b + 32,
                                1024 * pl_i:1024 * (pl_i + 1)],
                        in_=acc[32 * pl_i:32 * pl_i + 32, :])
            pair_allreduce(e3f, 3072)
            e3 = p0.tile([P, 3072], DT, tag="e3", name="e3")
            nc.vector.tensor_copy(out=e3[:, 0:1024], in_=e3f[:, 0:1024])
            nc.scalar.activation(out=e3[:, 1024:2048], in_=e3f[:, 1024:2048],
                                 func=ACT.Identity)
            nc.gpsimd.tensor_copy(out=e3[:, 2048:3072], in_=e3f[:, 2048:3072])

            # ---------- h -> hT ----------
            hT = p0.tile([P, S], DT, tag="hT")
            for i in range(NT):
                hp = psv.tile([P, R], F32, tag="pvacc", name="hacc")
                for t in range(DT_T):
                    nc.tensor.matmul(out=hp[:],
                                     lhsT=nxT[t][:, i * P:(i + 1) * P],
                                     rhs=sc_b[:, t * P:(t + 1) * P],
                                     start=(t == 0), stop=(t == DT_T - 1))
                hb = sp.tile([P, R], DT, tag="h_b", name="h_b")
                nc.vector.tensor_copy(out=hb[:], in_=hp[:])
                htp = pst.tile([P, P], DT, tag="tpp", name="tpp")
                nc.tensor.transpose(out=htp[:], in_=hb[:], identity=idn)
                nc.gpsimd.tensor_copy(out=hT[:, i * P:(i + 1) * P], in_=htp[:])

            # ---------- K, Q, V ----------
            SCALE_Q = 1.0 / float(np.sqrt(DH))
            kT = [p4.tile([P, S], DT, tag=f"kT{t}", name=f"kT{t}")
                  for t in range(DT_T)]
            qT = [p4.tile([P, SQ], DT, tag=f"qT{t}", name=f"qT{t}")
                  for t in range(DT_T)]
            vext = [p4.tile([P, H * (DH + 1)], DT, tag=f"vx{i}", name=f"vx{i}")
                    for i in range(NT)]
            for t in range(DT_T):
                kp = psa.tile([P, S], F32, tag="acc", name="acc")
                for j in range(2):
                    nc.tensor.matmul(out=kp[:, j * 512:(j + 1) * 512],
                                     lhsT=e3[:, 1024 + t * P:1024 + t * P + P],
                                     rhs=hT[:, j * 512:(j + 1) * 512],
                                     start=True, stop=True)
                nc.scalar.activation(out=kT[t][:], in_=kp[:], func=ACT.Identity)
                qp = psv.tile([P, SQ], F32, tag="pvacc", name="qacc")
                nc.tensor.matmul(out=qp[:], lhsT=e3[:, t * P:t * P + P],
                                 rhs=hT[:, 0:SQ], start=True, stop=True)
                nc.vector.tensor_scalar(out=qT[t][:], in0=qp[:],
                                        scalar1=SCALE_Q, scalar2=None,
                                        op0=ALU.mult)
            for i in range(NT):
                vp = psa.tile([P, D], F32, tag="acc", name="acc")
                for j in range(2):
                    nc.tensor.matmul(
                        out=vp[:, j * 512:(j + 1) * 512],
                        lhsT=hT[:, i * P:(i + 1) * P],
                        rhs=e3[:, 2048 + j * 512:2048 + (j + 1) * 512],
                        start=True, stop=True)
                vv = vext[i][:].rearrange("p (hh c) -> p hh c", c=DH + 1)
                nc.vector.tensor_copy(
                    out=vv[:, :, 0:DH],
                    in_=vp[:].rearrange("p (hh c) -> p hh c", c=DH))
                nc.gpsimd.memset(vv[:, :, DH:DH + 1], 1.0)
        # ph0 (nxT, scf, e3f, lhs3) released

        # ---------- attention ----------
        # q-slot s covers local q-tile s; key positions {0..s} u {4..7}.
        # position j==s gets the constant tri mask via PE; positions 4..7 get
        # the per-core bias column (0 or -1e9) folded into the exp.
        attnT = [p4.tile([P, SQ], DT, tag=f"at{t}", name=f"at{t}")
                 for t in range(DT_T)]
        for hd in range(H):
            t4 = hd // 2
            hs = (hd % 2) * DH
            for s in range(QT):
                poss = list(range(s + 1)) + [4, 5, 6, 7]
                sps = psa.tile([P, 1024], F32, tag="acc", name="sacc")
                for j in poss:
                    osl = sps[:, j * P:(j + 1) * P]
                    nc.tensor.matmul(out=osl,
                                     lhsT=kT[t4][hs:hs + DH, j * P:(j + 1) * P],
                                     rhs=qT[t4][hs:hs + DH, s * P:(s + 1) * P],
                                     start=True, stop=(j != s))
                    if j == s:
                        nc.tensor.matmul(out=osl, lhsT=idn, rhs=tri,
                                         start=False, stop=True)
                pt = sp.tile([P, 1024], DT, tag="p_tile", name="p_tile")
                nc.scalar.activation(out=pt[:, 0:(s + 1) * P],
                                     in_=sps[:, 0:(s + 1) * P], func=ACT.Exp)
                nc.scalar.activation(out=pt[:, 512:1024],
                                     in_=sps[:, 512:1024], func=ACT.Exp,
                                     bias=negc)
                po = psv.tile([DH + 1, P], F32, tag="pvacc", name="pvacc")
                for n, j in enumerate(poss):
                    nc.tensor.matmul(
                        out=po[:],
                        lhsT=vext[j][:, hd * (DH + 1):(hd + 1) * (DH + 1)],
                        rhs=pt[:, j * P:(j + 1) * P],
                        start=(n == 0), stop=(n == len(poss) - 1))
                rec = sp.tile([1, P], F32, tag="rec", name="rec")
                nc.vector.reciprocal(rec[:], po[DH:DH + 1, :])
                recB = sp.tile([DH, P], F32, tag="recB", name="recB")
                nc.gpsimd.partition_broadcast(recB[:], rec[:])
                nc.vector.tensor_tensor(
                    out=attnT[t4][hs:hs + DH, s * P:(s + 1) * P],
                    in0=po[0:DH, :], in1=recB[:], op=ALU.mult)

        # ---------- WO + residual (into xa) ----------
        for i in range(QT):
            wp = psa.tile([P, D], F32, tag="acc", name="acc")
            for j in range(2):
                for t in range(DT_T):
                    nc.tensor.matmul(
                        out=wp[:, j * 512:(j + 1) * 512],
                        lhsT=attnT[t][:, i * P:(i + 1) * P],
                        rhs=wota[:, 1024 * t + 512 * j:1024 * t + 512 * (j + 1)],
                        start=(t == 0), stop=(t == DT_T - 1))
            nc.vector.tensor_tensor(out=xa[:, i * 1024:(i + 1) * 1024],
                                    in0=wp[:], in1=xa[:, i * 1024:(i + 1) * 1024],
                                    op=ALU.add)

        ctx4.close()

        # ---------- memory block ----------
        with tc.tile_pool(name="ph6", bufs=1) as p6:
            nx2T = [p6.tile([P, SQ], DT, tag=f"n2T{t}", name=f"n2T{t}")
                    for t in range(DT_T)]
            kkt = p6.tile([KR, NK], DT, tag="kkt", name="kkt")
            nc.sync.dma_start(out=kkt[:], in_=I["kkt"][:])
            load_gb(2, 3)
            for i in range(QT):
                nx2_i = layernorm_tile(xa[:, i * 1024:(i + 1) * 1024], sp,
                                       "nx2")
                for t in range(DT_T):
                    tp = pst.tile([P, P], DT, tag="tpp", name="tpp")
                    nc.tensor.transpose(out=tp[:],
                                        in_=nx2_i[:, t * P:(t + 1) * P],
                                        identity=idn)
                    eng = nc.gpsimd if (t % 2 == 0) else nc.vector
                    eng.tensor_copy(out=nx2T[t][:, i * P:(i + 1) * P],
                                    in_=tp[:])

            mwp_ps = psv.tile([1, NEXP], F32, tag="pvacc", name="pvacc")
            for i in range(QT):
                pr = psa.tile([P, NEXP], F32, tag="acc", name="acc")
                for t in range(DT_T):
                    nc.tensor.matmul(out=pr[:],
                                     lhsT=nx2T[t][:, i * P:(i + 1) * P],
                                     rhs=wct[:, 320 * t + 256:320 * t + 320],
                                     start=(t == 0), stop=(t == DT_T - 1))
                prefm = sp.tile([P, NEXP], DT, tag="prefm", name="prefm")
                softmax_blocks(pr[:], prefm[:], 1, NEXP)
                nc.tensor.matmul(out=mwp_ps[:], lhsT=impa[:, i:i + 1],
                                 rhs=prefm[:], start=(i == 0),
                                 stop=(i == QT - 1))

            mwrow = p6.tile([1, NEXP], F32, tag="mwrow", name="mwrow")
            if use_cc:
                mwr = sp.tile([1, NEXP], F32, tag="mwr", name="mwr")
                nc.vector.tensor_copy(out=mwr[:], in_=mwp_ps[:])
                ccp = psa.tile([B, NEXP], F32, tag="acc", name="acc")
                nc.tensor.matmul(out=ccp[:], lhsT=bselr, rhs=mwr[:],
                                 start=True, stop=True)
                cc_sb = sp.tile([B, NEXP], F32, tag="cc_sb", name="cc_sb")
                nc.vector.tensor_copy(out=cc_sb[:], in_=ccp[:])
                cc_in = dr.tile([B, NEXP], F32)
                cc_out = dr.tile([B, NEXP], F32)
                nc.gpsimd.dma_start(out=cc_in[:], in_=cc_sb[:])
                nc.gpsimd.collective_compute(
                    "AllReduce", ALU.add,
                    replica_groups=[list(range(N_CORES))],
                    ins=[cc_in.opt()], outs=[cc_out.opt()])
                cc_res = sp.tile([B, NEXP], F32, tag="cc_res", name="cc_res")
                nc.gpsimd.dma_start(out=cc_res[:], in_=cc_out[:])
                mwf = psa.tile([1, NEXP], F32, tag="acc", name="acc")
                nc.tensor.matmul(out=mwf[:], lhsT=bselc, rhs=cc_res[:],
                                 start=True, stop=True)
                nc.vector.tensor_copy(out=mwrow[:], in_=mwf[:])
            else:
                nc.vector.tensor_copy(out=mwrow[:], in_=mwp_ps[:])
            st = sp.tile([1, 1], F32, tag="wn_st", name="wn_st")
            nc.vector.tensor_reduce(out=st[:], in_=mwrow[:], axis=AX,
                                    op=ALU.add)
            nc.vector.tensor_scalar(out=st[:], in0=st[:], scalar1=1e-8,
                                    scalar2=None, op0=ALU.add)
            nc.vector.reciprocal(st[:], st[:])
            nc.vector.tensor_scalar(out=mwrow[:], in0=mwrow[:], scalar1=st[:],
                                    scalar2=None, op0=ALU.mult)
            mwrow_cp = sp.tile([1, NEXP], F32, tag="mwr2", name="mwr2")
            nc.vector.tensor_copy(out=mwrow_cp[:], in_=mwrow[:])
            mwt = pst.tile([NEXP, 1], F32, tag="tpp", name="mwt")
            nc.tensor.transpose(out=mwt[:], in_=mwrow_cp[:], identity=one1)
            mwcol = p6.tile([NEXP, 1], F32, tag="mwcol", name="mwcol")
            nc.vector.tensor_copy(out=mwcol[:], in_=mwt[:])

            scmf = p6.tile([P, 1024], F32, tag="scmf", name="scmf")
            combine_cn(mwcol[0:NLOC, 0:1], cn2, scmf)
            pair_allreduce(scmf, 1024)
            scm_b = p6.tile([P, 1024], DT, tag="scm_b", name="scm_b")
            nc.vector.tensor_copy(out=scm_b[:], in_=scmf[:])

            # QmT [r, sq]
            qmp = psv.tile([P, SQ], F32, tag="pvacc", name="pvacc")
            for t in range(DT_T):
                nc.tensor.matmul(out=qmp[:], lhsT=scm_b[:, t * P:(t + 1) * P],
                                 rhs=nx2T[t][:], start=(t == 0),
                                 stop=(t == DT_T - 1))
            qmT = p6.tile([P, SQ], DT, tag="qmT")
            nc.vector.tensor_scalar(out=qmT[:], in0=qmp[:],
                                    scalar1=1.0 / float(np.sqrt(KR)),
                                    scalar2=None, op0=ALU.mult)

            idx_all = p6.tile([P, QT * TOPK], U32, tag="idx_all",
                              name="idx_all")
            w8_all = p6.tile([P, QT * TOPK], F32, tag="w8_all", name="w8_all")
            for i in range(QT):
                ks = p6.tile([P, NK], F32, tag="ks_sb", name="ks_sb")
                for j in range(NK // 512):
                    ksp = psa.tile([P, 512], F32, tag="acc", name="acc")
                    nc.tensor.matmul(out=ksp[:],
                                     lhsT=qmT[:, i * P:(i + 1) * P],
                                     rhs=kkt[:, j * 512:(j + 1) * 512],
                                     start=True, stop=True)
                    eng = nc.scalar if (j % 2 == 0) else nc.vector
                    if eng is nc.scalar:
                        nc.scalar.activation(out=ks[:, j * 512:(j + 1) * 512],
                                             in_=ksp[:], func=ACT.Identity)
                    else:
                        nc.vector.tensor_copy(out=ks[:, j * 512:(j + 1) * 512],
                                              in_=ksp[:])
                tv = sp.tile([P, TOPK], F32, tag="tv", name="tv")
                nc.vector.max(out=tv[:], in_=ks[:])
                nc.vector.max_index(out=idx_all[:, i * TOPK:(i + 1) * TOPK],
                                    in_max=tv[:], in_values=ks[:])
                st8 = sp.tile([P, 2], F32, tag="st8", name="st8")
                nm = st8[:, 0:1]; se8 = st8[:, 1:2]
                nc.vector.tensor_scalar(out=nm, in0=tv[:, 0:1], scalar1=-1.0,
                                        scalar2=None, op0=ALU.mult)
                w8 = sp.tile([P, TOPK], F32, tag="w8", name="w8")
                nc.scalar.activation(out=w8[:], in_=tv[:], func=ACT.Exp,
                                     bias=nm, accum_out=se8)
                nc.vector.reciprocal(se8, se8)
                nc.vector.tensor_scalar(out=w8_all[:, i * TOPK:(i + 1) * TOPK],
                                        in0=w8[:], scalar1=se8, scalar2=None,
                                        op0=ALU.mult)

            for i in range(QT):
                acc = p6.tile([P, D], F32, tag="mem_acc", name="mem_acc")
                for k in range(TOPK):
                    g = i * TOPK + k
                    gt = sp.tile([P, D], DT, tag="gath", name="gath", bufs=2)
                    nc.gpsimd.indirect_dma_start(
                        out=gt[:], out_offset=None, in_=I["kv"][:],
                        in_offset=bass.IndirectOffsetOnAxis(
                            ap=idx_all[:, g:g + 1], axis=0))
                    prev = xa[:, i * 1024:(i + 1) * 1024] if k == 0 else acc[:]
                    eng = nc.vector if (k % 2 == 0) else nc.gpsimd
                    eng.scalar_tensor_tensor(
                        out=acc[:], in0=gt[:], scalar=w8_all[:, g:g + 1],
                        in1=prev, op0=ALU.mult, op1=ALU.add)
                nc.sync.dma_start(out=o[i * P:(i + 1) * P, :], in_=acc[:])


# ---------------- PJRT SPMD runner (persistent jit) ----------------

class SpmdRunner:
    def __init__(self, nc, n_cores):
        import jax
        from jax.sharding import Mesh, PartitionSpec
        from jax.experimental.shard_map import shard_map
        from concourse import bass2jax
        bass2jax.install_neuronx_cc_hook()
        self.jax = jax
        self.nc = nc
        self.n_cores = n_cores
        partition_name = (nc.partition_id_tensor.name
                          if nc.partition_id_tensor else None)
        in_names, out_names, out_avals, zero_outs = [], [], [], []
        for alloc in nc.m.functions[0].allocations:
            if not isinstance(alloc, mybir.MemoryLocationSet):
                continue
            name = alloc.memorylocations[0].name
            if alloc.kind == "ExternalInput":
                if name != partition_name:
                    in_names.append(name)
            elif alloc.kind == "ExternalOutput":
                shape = tuple(alloc.tensor_shape)
                dtype = mybir.dt.np(alloc.dtype)
                out_names.append(name)
                out_avals.append(jax.core.ShapedArray(shape, dtype))
                zero_outs.append(np.zeros(shape, dtype))
        self.n_params = len(in_names)
        self.in_names = list(in_names)
        self.out_names = out_names
        self.out_avals = out_avals
        self.zero_outs = zero_outs
        all_in = in_names + out_names + ([partition_name] if partition_name
                                         else [])

        def _body(*args):
            operands = list(args)
            if partition_name is not None:
                operands.append(bass2jax.partition_id_tensor())
            outs = bass2jax._bass_exec_p.bind(
                *operands, out_avals=tuple(out_avals), in_names=tuple(all_in),
                out_names=tuple(out_names), lowering_input_output_aliases=(),
                sim_require_finite=True, sim_require_nnan=True, nc=nc)
            return tuple(outs)

        devices = jax.devices()[:n_cores]
        self.mesh = Mesh(np.asarray(devices), ("core",))
        nspec = self.n_params + len(out_names)
        self.sharded = jax.jit(
            shard_map(_body, mesh=self.mesh,
                      in_specs=(PartitionSpec("core"),) * nspec,
                      out_specs=(PartitionSpec("core"),) * len(out_names),
                      check_rep=False),
            keep_unused=True)

    def concat_inputs(self, in_maps):
        per_core = [[np.asarray(m[n]) for n in self.in_names] for m in in_maps]
        cat = [np.concatenate([per_core[c][i] for c in range(self.n_cores)],
                              axis=0) for i in range(self.n_params)]
        cat += [np.zeros((self.n_cores * z.shape[0], *z.shape[1:]), z.dtype)
                for z in self.zero_outs]
        return cat

    def run(self, in_maps):
        out_arrs = self.sharded(*self.concat_inputs(in_maps))
        self.jax.block_until_ready(out_arrs)
        return [
            {n: np.asarray(out_arrs[i]).reshape(
                self.n_cores, *self.out_avals[i].shape)[c]
             for i, n in enumerate(self.out_names)}
            for c in range(self.n_cores)
        ]


# ---------------- host side ----------------

_RUNNER = None


def _make_inputs(x, importance, mask, compress_neurons, expand_pool,
                 knowledge_K, knowledge_V, Wc, WQ, WK, WV, Wm, WO,
                 g1, b1, g2, b2):
    ndt = np_bdt()
    f = lambda a: np.asarray(a, np.float32)
    cn = f(compress_neurons)
    pl = f(expand_pool)
    wstack = np.concatenate([f(Wc), f(WQ), f(WK), f(WV), f(Wm)], axis=0)
    wot = np.ascontiguousarray(f(WO).T)  # [D, D] = WO.T
    wotr = np.empty((P, NT * 1024), np.float32)
    for t in range(NT):
        wotr[:, 1024 * t:1024 * (t + 1)] = wot[128 * t:128 * (t + 1), :]
    kkt = np.ascontiguousarray(f(knowledge_K).T).astype(ndt)
    kv = f(knowledge_V).astype(ndt)

    # aux (core-independent parts)
    auxb = np.zeros((P, AB_W), np.float32)
    auxb[:, AB_IDN:AB_IDN + P] = np.eye(P)
    auxb[:, AB_BMS:AB_BMS + 32] = (
        (np.arange(P)[:, None] // 4) == np.arange(32)[None, :])
    ktri = np.arange(P)
    auxb[:, AB_TRI:AB_TRI + P] = np.where(
        ktri[None, :] >= ktri[:, None], 0.0, NEG)  # tri[k, q]
    auxb = auxb.astype(ndt)

    gb = np.stack([f(g1), f(b1), f(g2), f(b2)]).reshape(4, D)

    x = f(x); importance = f(importance)
    in_maps = []
    for c in range(N_CORES):
        b, hf = c // 2, c % 2
        qr = np.arange(hf * SQ, hf * SQ + SQ)
        rest = np.arange((1 - hf) * SQ, (1 - hf) * SQ + SQ)
        perm = np.concatenate([qr, rest])
        eperm = (np.arange(NEXP) + NLOC * hf) % NEXP  # local experts first

        m = {}
        m["x"] = np.ascontiguousarray(x[b][perm])
        impc = importance[b][perm].reshape(NT, P).T  # [p, i]
        m["imp"] = np.ascontiguousarray(impc).astype(ndt)

        # wct: [128, 8*320]; block order [Wc|WQ|WK|WV|Wm], experts permuted
        wp_ = wstack.reshape(5, NEXP, D)[:, eperm, :].reshape(5 * NEXP, D)
        wctT = wp_.T  # [D, 320]
        wcth = np.empty((P, NT * 320), np.float32)
        for t in range(NT):
            wcth[:, 320 * t:320 * (t + 1)] = wctT[128 * t:128 * (t + 1), :]
        m["wct"] = np.ascontiguousarray(wcth).astype(ndt)
        m["wot"] = wotr.astype(ndt)
        m["kkt"] = kkt
        m["kv"] = kv
        m["gb"] = gb

        # cnb[b4, g, p, 128t + r] = cn[e(g,p), 128t + 32*b4 + p//4, r]
        loc = eperm[:NLOC]
        cl = cn[loc]                      # [32, D, R]
        clr = cl.reshape(NLOC, 8, 128, R) # [n, t, dsub, r]
        # dsub = 32*b4 + p//4 ; partition p = 4*(p//4) + n%4
        cnb = np.empty((4, GLOC, P, 1024), np.float32)
        for b4 in range(4):
            blk = clr[:, :, 32 * b4:32 * (b4 + 1), :]   # [n, t, 32, r]
            for g in range(GLOC):
                for e in range(4):
                    n = 4 * g + e
                    # partition p = 4*m + e (m = dsub idx), free = 128t + r
                    cnb[b4, g, e::4, :] = blk[n].transpose(1, 0, 2).reshape(
                        32, 8 * 128)
        m["cnb"] = np.ascontiguousarray(cnb).astype(ndt)

        pll = pl[loc]                     # [32, R, D]
        plb = np.empty((4, GLOC, P, 1024), np.float32)
        for b4 in range(4):
            blk = pll[:, 32 * b4:32 * (b4 + 1), :]      # [n, 32, D]
            for g in range(GLOC):
                for e in range(4):
                    plb[b4, g, e::4, :] = blk[4 * g + e]
        m["plb"] = np.ascontiguousarray(plb).astype(ndt)

        auxf = np.zeros((P, AF_W), np.float32)
        auxf[:NLOC, AF_A4:AF_A4 + P] = (
            (np.arange(NLOC)[:, None] % 4) == (np.arange(P)[None, :] % 4))
        auxf[:NLOC, AF_B8:AF_B8 + GLOC] = (
            (np.arange(NLOC)[:, None] // 4) == np.arange(GLOC)[None, :])
        onehot = np.zeros(B, np.float32); onehot[b] = 1.0
        auxf[0:1, AF_BSELR:AF_BSELR + B] = onehot[None, :]
        auxf[0:B, AF_BSELC:AF_BSELC + 1] = onehot[:, None]
        auxf[:, AF_NEGC] = NEG if hf == 0 else 0.0
        auxf[0, AF_ONE] = 1.0
        m["auxf"] = auxf
        m["auxb"] = auxb
        in_maps.append(m)
    return in_maps


def _get_runner():
    global _RUNNER
    if _RUNNER is None:
        nc = build_nc(use_cc=True)
        _RUNNER = SpmdRunner(nc, N_CORES)
    return _RUNNER


def kernel(**inputs):
    r = _get_runner()
    in_maps = _make_inputs(**inputs)
    res = r.run(in_maps)
    out = np.empty((B, S, D), np.float32)
    for c in range(N_CORES):
        b, hf = c // 2, c % 2
        out[b, hf * SQ:(hf + 1) * SQ] = res[c]["o"]
    return out
